# revision 1
# baseline (speedup 1.0000x reference)
"""Trainium2 Bass kernel for nn_JambaAttentionDecoderLayer (8 NeuronCores).

Sharding:
  - Attention: sequence-parallel. Core c owns tokens [256c, 256c+256). Host
    feeds each core its transposed halo window xT[1024, 768] (512-token halo
    for the sliding-window attention) plus a precomputed additive mask, so
    attention needs no collectives.
  - MoE: expert-parallel. Core c owns expert c (full 2816 intermediate dim,
    bf16 weights). h is AllGathered (bf16) + router logits AllGathered (f32,
    computed locally in f32 so every core sees bit-identical logits -> no
    cross-core top-k disagreement). Each core computes its expert densely
    over all 2048 tokens, scales by its gating column, and a ReduceScatter
    sums the 8 partials; core c ends up with output rows [256c, 256c+256).
"""

import os
import sys

import numpy as np

sys.path.insert(0, "/opt/trn_rl_repo")

import ml_dtypes  # noqa: E402

import concourse.bass as bass  # noqa: E402
import concourse.tile as tile  # noqa: E402
from concourse import bacc, mybir  # noqa: E402
from concourse.bass_utils import run_bass_kernel_spmd  # noqa: E402

F32 = mybir.dt.float32
F32R = mybir.dt.float32r
BF16 = mybir.dt.bfloat16

B, S, H = 1, 2048, 1024
NH, NKV, HD = 16, 4, 64
I, E, K = 2816, 8, 2
WIN = 512
EPS = 1e-5
NCORES = 8
SC = S // NCORES          # 256 tokens per core
HALO = SC + WIN           # 768 key/value window per core
HC = H // 128             # 8 H chunks
ICH = I // 128            # 22 chunks of I
TBLK = 512                # ffn token block
NBLK = S // TBLK          # 4

_CACHE = {}


def _build_module():
    nc = bacc.Bacc("TRN2", target_bir_lowering=False, debug=False, num_devices=NCORES)

    # ---- kernel I/O ----
    xT = nc.dram_tensor("xT", [H, HALO], F32, kind="ExternalInput")
    maskT = nc.dram_tensor("maskT", [HALO, SC], F32, kind="ExternalInput")
    wqT = nc.dram_tensor("wqT", [H, NH * HD], F32, kind="ExternalInput")
    wkT = nc.dram_tensor("wkT", [H, NKV * HD], F32, kind="ExternalInput")
    wvT = nc.dram_tensor("wvT", [H, NKV * HD], F32, kind="ExternalInput")
    woT = nc.dram_tensor("woT", [NH * HD, H], F32, kind="ExternalInput")
    routerT = nc.dram_tensor("routerT", [H, E], F32, kind="ExternalInput")
    w1T = nc.dram_tensor("w1T", [H, 2 * I], BF16, kind="ExternalInput")
    w2T = nc.dram_tensor("w2T", [I, H], BF16, kind="ExternalInput")
    cidx = nc.dram_tensor("cidx", [128, 2], F32, kind="ExternalInput")
    identf = nc.dram_tensor("identf", [128, 128], F32, kind="ExternalInput")
    onesf = nc.dram_tensor("onesf", [128, 128], F32, kind="ExternalInput")
    onesb = nc.dram_tensor("onesb", [128, 1], BF16, kind="ExternalInput")
    epsi = nc.dram_tensor("epsi", [1, 1], F32, kind="ExternalInput")

    res2T = nc.dram_tensor("res2T", [H, SC], F32, kind="ExternalOutput")
    moe_out = nc.dram_tensor("moe_out", [SC, H], F32, kind="ExternalOutput")

    # ---- internal DRAM (collective bounce buffers) ----
    ag1_in = nc.dram_tensor("ag1_in", [H, SC], BF16)
    ag1_out = nc.dram_tensor("ag1_out", [NCORES * H, SC], BF16, addr_space="Shared")
    ag2_in = nc.dram_tensor("ag2_in", [E, SC], F32)
    ag2_out = nc.dram_tensor("ag2_out", [NCORES * E, SC], F32, addr_space="Shared")
    outAcc = nc.dram_tensor("outAcc", [S, H], F32)
    rs_out = nc.dram_tensor("rs_out", [SC, H], F32)

    with tile.TileContext(nc) as tc:
        with (
            tc.tile_pool(name="const", bufs=1) as const,
            tc.tile_pool(name="pers", bufs=1) as pers,
            tc.tile_pool(name="work", bufs=2) as work,
            tc.tile_pool(name="wstream", bufs=4) as wstream,
        ):
            ident = const.tile([128, 128], F32)
            nc.sync.dma_start(ident[:], identf[:, :])
            eps_t = const.tile([1, 1], F32)
            nc.sync.dma_start(eps_t[:], epsi[:, :])
            ones_bf = const.tile([128, 1], BF16)
            nc.sync.dma_start(ones_bf[:], onesb[:, :])
            onesf_sb = const.tile([128, 128], F32)
            nc.sync.dma_start(onesf_sb[:], onesf[:, :])
            ones_f = onesf_sb[:, 0:1]
            ones_row = onesf_sb[0:1, :]
            cidx_sb = const.tile([128, 2], F32)
            nc.sync.dma_start(cidx_sb[:], cidx[:, :])

            hT_bf = pers.tile([128, HC, SC], BF16)
            wcol = pers.tile([128, S // 128], F32)

            attn_cm = tc.tile_pool(name="attn", bufs=1)
            attn = attn_cm.__enter__()

            # ---------- attention ----------
            xT_sb = attn.tile([128, HC, HALO], F32)
            nc.sync.dma_start(
                xT_sb[:], xT[:, :].rearrange("(k p) f -> p k f", p=128)
            )
            maskT_sb = attn.tile([128, HALO // 128, SC], F32)
            nc.sync.dma_start(
                maskT_sb[:], maskT[:, :].rearrange("(k p) f -> p k f", p=128)
            )

            # rms statistics over H (partition axis) via ones-matmul on x^2
            ps_qkv_cm = tc.tile_pool(name="ps_qkv", bufs=1, space="PSUM")
            ps_qkv = ps_qkv_cm.__enter__()
            ss_ps = ps_qkv.tile([1, HALO], F32, space="PSUM")
            for k in range(HC):
                sq = work.tile([128, HALO], BF16, tag="sq")
                nc.vector.tensor_mul(sq[:], xT_sb[:, k, :], xT_sb[:, k, :])
                for lo, hi in ((0, 512), (512, HALO)):
                    nc.tensor.matmul(
                        out=ss_ps[:, lo:hi],
                        lhsT=ones_bf[:],
                        rhs=sq[:, lo:hi],
                        start=(k == 0),
                        stop=(k == HC - 1),
                    )
            inv1 = attn.tile([1, HALO], F32)
            nc.scalar.activation(
                out=inv1[:], in_=ss_ps[:],
                func=mybir.ActivationFunctionType.Sqrt,
                bias=eps_t[:], scale=1.0 / H,
            )
            nc.vector.reciprocal(inv1[:], inv1[:])
            inv1bc = attn.tile([128, HALO], F32)
            bc1 = ps_qkv.tile([128, HALO], F32, space="PSUM", tag="kps", name="bc1")
            for lo, hi in ((0, 512), (512, HALO)):
                nc.tensor.matmul(out=bc1[:, lo:hi], lhsT=ones_row,
                                 rhs=inv1[:, lo:hi], start=True, stop=True)
            nc.vector.tensor_copy(inv1bc[:], bc1[:])
            # per-token inv on the partition axis (for V token-major scaling)
            invT_sb = attn.tile([128, HALO // 128], F32)
            for mt in range(HALO // 128):
                tpv = ps_qkv.tile([128, 128], F32, space="PSUM", tag="tpv")
                nc.tensor.transpose(
                    out=tpv[:], in_=inv1bc[:, 128 * mt:128 * (mt + 1)],
                    identity=ident[:],
                )
                nc.vector.tensor_copy(invT_sb[:, mt:mt + 1], tpv[:, 0:1])

            # K/V/Q projections (f32r matmuls, weights streamed).
            # Heads live on partitions [0,64) so score matmuls contract at a
            # uniform base partition: kT64 [64, kv_head, keys], qT64 [64, head, q].
            kT64 = attn.tile([64, NKV, HALO], F32)
            for m in range(2):
                kps = ps_qkv.tile([128, HALO], F32, space="PSUM", tag="kps")
                for k in range(HC):
                    wt = wstream.tile([128, 128], F32, tag="wk")
                    nc.sync.dma_start(
                        wt[:], wkT[128 * k:128 * (k + 1), 128 * m:128 * (m + 1)]
                    )
                    for lo, hi in ((0, 512), (512, HALO)):
                        nc.tensor.matmul(
                            out=kps[:, lo:hi],
                            lhsT=wt[:],
                            rhs=xT_sb[:, k, lo:hi],
                            start=(k == 0), stop=(k == HC - 1),
                        )
                ktmp = work.tile([128, HALO], F32, tag="ktmp")
                nc.vector.tensor_mul(ktmp[:], kps[:], inv1bc[:])
                nc.sync.dma_start(kT64[0:64, 2 * m, :], ktmp[0:64, :])
                nc.sync.dma_start(kT64[0:64, 2 * m + 1, :], ktmp[64:128, :])

            qT64 = attn.tile([64, NH, SC], F32)
            for m in range(HC):
                qps = ps_qkv.tile([128, SC], F32, space="PSUM", tag="qps")
                for k in range(HC):
                    wt = wstream.tile([128, 128], F32, tag="wq")
                    nc.sync.dma_start(
                        wt[:], wqT[128 * k:128 * (k + 1), 128 * m:128 * (m + 1)]
                    )
                    nc.tensor.matmul(
                        out=qps[:],
                        lhsT=wt[:],
                        rhs=xT_sb[:, k, WIN:HALO],
                        start=(k == 0), stop=(k == HC - 1),
                    )
                qtmp = work.tile([128, SC], F32, tag="qtmp")
                nc.vector.tensor_mul(qtmp[:], qps[:], inv1bc[:, WIN:HALO])
                nc.sync.dma_start(qT64[0:64, 2 * m, :], qtmp[0:64, :])
                nc.sync.dma_start(qT64[0:64, 2 * m + 1, :], qtmp[64:128, :])

            vtok = attn.tile([128, HALO // 128, NKV * HD], F32)
            for mt in range(HALO // 128):
                vps = ps_qkv.tile([128, NKV * HD], F32, space="PSUM", tag="vps")
                for k in range(HC):
                    wt = wstream.tile([128, NKV * HD], F32, tag="wv")
                    nc.sync.dma_start(wt[:], wvT[128 * k:128 * (k + 1), :])
                    nc.tensor.matmul(
                        out=vps[:],
                        lhsT=xT_sb[:, k, 128 * mt:128 * (mt + 1)],
                        rhs=wt[:],
                        start=(k == 0), stop=(k == HC - 1),
                    )
                nc.vector.tensor_scalar_mul(
                    out=vtok[:, mt, :], in0=vps[:],
                    scalar1=invT_sb[:, mt:mt + 1],
                )
            ps_qkv_cm.__exit__(None, None, None)

            # per-head attention, all transposed layouts
            ps_att_cm = tc.tile_pool(name="ps_att", bufs=1, space="PSUM")
            ps_att = ps_att_cm.__enter__()
            oT64 = attn.tile([64, NH, SC], F32)
            KT = HALO // 128  # 6 key tiles
            for h in range(NH):
                g = h // (NH // NKV)
                l_ps = ps_att.tile([1, SC], F32, space="PSUM", tag="l_ps")
                o_ps = ps_att.tile([64, SC], F32, space="PSUM", tag="o_ps")
                for kt in range(KT):
                    s_ps = ps_att.tile([128, SC], F32, space="PSUM", tag="s_ps")
                    nc.tensor.matmul(
                        out=s_ps[:],
                        lhsT=kT64[0:64, g, 128 * kt:128 * (kt + 1)],
                        rhs=qT64[0:64, h, :],
                        start=True, stop=True,
                    )
                    pt = work.tile([128, SC], F32, tag="pt")
                    nc.vector.tensor_add(pt[:], s_ps[:], maskT_sb[:, kt, :])
                    nc.scalar.activation(
                        out=pt[:], in_=pt[:],
                        func=mybir.ActivationFunctionType.Exp,
                    )
                    nc.tensor.matmul(
                        out=l_ps[:],
                        lhsT=ones_f,
                        rhs=pt[:],
                        start=(kt == 0), stop=(kt == KT - 1),
                    )
                    nc.tensor.matmul(
                        out=o_ps[:],
                        lhsT=vtok[:, kt, 64 * g:64 * (g + 1)],
                        rhs=pt[:],
                        start=(kt == 0), stop=(kt == KT - 1),
                    )
                linv = work.tile([1, SC], F32, tag="linv")
                nc.vector.reciprocal(linv[:], l_ps[:])
                lbcp = ps_att.tile([64, SC], F32, space="PSUM", tag="lbcp")
                nc.tensor.matmul(out=lbcp[:], lhsT=ones_row[:, 0:64],
                                 rhs=linv[:], start=True, stop=True)
                lbc = work.tile([64, SC], F32, tag="lbc")
                nc.vector.tensor_copy(lbc[:], lbcp[:])
                nc.vector.tensor_mul(oT64[0:64, h, :], o_ps[:], lbc[:])
            ps_att_cm.__exit__(None, None, None)

            # o_proj + residual -> x2T ; also write res2T output
            ps_h_cm = tc.tile_pool(name="ps_h", bufs=1, space="PSUM")
            ps_h = ps_h_cm.__enter__()
            x2T = attn.tile([128, HC, SC], F32)
            for hc in range(HC):
                aps = ps_h.tile([128, SC], F32, space="PSUM", tag="aps")
                for oh in range(NH):
                    wt = wstream.tile([64, 128], F32, tag="wo")
                    nc.sync.dma_start(
                        wt[:], woT[64 * oh:64 * (oh + 1), 128 * hc:128 * (hc + 1)]
                    )
                    nc.tensor.matmul(
                        out=aps[:],
                        lhsT=wt[:],
                        rhs=oT64[0:64, oh, :],
                        start=(oh == 0), stop=(oh == NH - 1),
                    )
                nc.vector.tensor_add(x2T[:, hc, :], aps[:], xT_sb[:, hc, WIN:HALO])
            nc.sync.dma_start(
                res2T[:, :].rearrange("(k p) f -> p k f", p=128), x2T[:]
            )

            # h = x2 * rsqrt(mean(x2^2)+eps)   (ln2 folded into router/w1 on host)
            ss2_ps = ps_h.tile([1, SC], F32, space="PSUM", tag="ss2")
            for k in range(HC):
                sq2 = work.tile([128, SC], BF16, tag="sq2")
                nc.vector.tensor_mul(sq2[:], x2T[:, k, :], x2T[:, k, :])
                nc.tensor.matmul(
                    out=ss2_ps[:], lhsT=ones_bf[:], rhs=sq2[:],
                    start=(k == 0), stop=(k == HC - 1),
                )
            inv2 = attn.tile([1, SC], F32)
            nc.scalar.activation(
                out=inv2[:], in_=ss2_ps[:],
                func=mybir.ActivationFunctionType.Sqrt,
                bias=eps_t[:], scale=1.0 / H,
            )
            nc.vector.reciprocal(inv2[:], inv2[:])
            inv2bc = attn.tile([128, SC], F32)
            bc2 = ps_h.tile([128, SC], F32, space="PSUM", tag="bc2")
            nc.tensor.matmul(out=bc2[:], lhsT=ones_row,
                             rhs=inv2[:], start=True, stop=True)
            nc.vector.tensor_copy(inv2bc[:], bc2[:])
            lg_ps = ps_h.tile([E, SC], F32, space="PSUM", tag="lg")
            for k in range(HC):
                hf = work.tile([128, SC], F32, tag="hf")
                nc.vector.tensor_mul(hf[:], x2T[:, k, :], inv2bc[:])
                nc.vector.tensor_copy(hT_bf[:, k, :], hf[:])
                rt = wstream.tile([128, E], F32, tag="rt")
                nc.sync.dma_start(rt[:], routerT[128 * k:128 * (k + 1), :])
                # exact f32 matmul for router logits (top-k tie consistency)
                nc.tensor.matmul(
                    out=lg_ps[:], lhsT=rt[:], rhs=hf[:],
                    start=(k == 0), stop=(k == HC - 1),
                )
            lg_sb = work.tile([E, SC], F32, tag="lgsb")
            nc.vector.tensor_copy(lg_sb[:], lg_ps[:])
            ps_h_cm.__exit__(None, None, None)
            nc.sync.dma_start(ag2_in[:, :], lg_sb[:])
            nc.sync.dma_start(
                ag1_in[:, :].rearrange("(k p) f -> p k f", p=128), hT_bf[:]
            )

            # ---------- collectives: gather h (bf16) + logits (f32) ----------
            nc.gpsimd.collective_compute(
                "AllGather", mybir.AluOpType.bypass,
                replica_groups=[list(range(NCORES))],
                ins=[ag1_in[:, :]], outs=[ag1_out[:, :]],
            )
            nc.gpsimd.collective_compute(
                "AllGather", mybir.AluOpType.bypass,
                replica_groups=[list(range(NCORES))],
                ins=[ag2_in[:, :]], outs=[ag2_out[:, :]],
            )

            attn_cm.__exit__(None, None, None)

            # ---------- routing (identical on every core) ----------
            ps_r_cm = tc.tile_pool(name="ps_r", bufs=2, space="PSUM")
            ps_r = ps_r_cm.__enter__()
            for r in range(NCORES):
                lgr = work.tile([E, SC], F32, tag="lgr")
                nc.sync.dma_start(lgr[:], ag2_out[E * r:E * (r + 1), :])
                for half in range(2):
                    t16 = 2 * r + half
                    tp = ps_r.tile([128, E], F32, space="PSUM", tag="tp")
                    nc.tensor.transpose(
                        out=tp[:], in_=lgr[:, 128 * half:128 * (half + 1)],
                        identity=ident[0:E, 0:E],
                    )
                    lgt = work.tile([128, E], F32, tag="lgt")
                    nc.vector.tensor_copy(lgt[:], tp[:])
                    srt = work.tile([128, 8], F32, tag="srt")
                    nc.vector.max(srt[:], lgt[:])
                    idx8 = work.tile([128, 8], mybir.dt.uint32, tag="idx8")
                    nc.vector.max_index(idx8[:], srt[:], lgt[:])
                    dd = work.tile([128, 8], F32, tag="dd")
                    nc.vector.tensor_scalar(
                        out=dd[:], in0=srt[:], scalar1=srt[:, 0:1],
                        scalar2=None, op0=mybir.AluOpType.subtract,
                    )
                    nc.scalar.activation(
                        out=dd[:], in_=dd[:],
                        func=mybir.ActivationFunctionType.Exp,
                    )
                    ssum = work.tile([128, 1], F32, tag="ssum")
                    nc.vector.tensor_reduce(
                        out=ssum[:], in_=dd[:],
                        axis=mybir.AxisListType.X, op=mybir.AluOpType.add,
                    )
                    nc.vector.reciprocal(ssum[:], ssum[:])
                    p01 = work.tile([128, 2], F32, tag="p01")
                    nc.vector.tensor_scalar_mul(
                        out=p01[:], in0=dd[:, 0:2], scalar1=ssum[:],
                    )
                    idf = work.tile([128, 2], F32, tag="idf")
                    nc.vector.tensor_copy(idf[:], idx8[:, 0:2])
                    eq = work.tile([128, 2], F32, tag="eq")
                    nc.vector.tensor_tensor(
                        out=eq[:], in0=idf[:],
                        in1=cidx_sb[:],
                        op=mybir.AluOpType.is_equal,
                    )
                    nc.vector.tensor_mul(eq[:], eq[:], p01[:])
                    nc.vector.tensor_reduce(
                        out=wcol[:, t16:t16 + 1], in_=eq[:],
                        axis=mybir.AxisListType.X, op=mybir.AluOpType.add,
                    )
            ps_r_cm.__exit__(None, None, None)

            # ---------- dense expert FFN over all tokens ----------
            ps_f_cm = tc.tile_pool(name="ps_f", bufs=1, space="PSUM")
            ps_f = ps_f_cm.__enter__()
            ag1_3d = ag1_out[:, :].rearrange("(r k p) f -> r k p f", p=128, k=HC)
            for nblk in range(NBLK):
                hT_blk = pers.tile([128, HC, TBLK], BF16, tag="hT_blk")
                for k in range(HC):
                    for j in range(TBLK // SC):
                        r = (TBLK // SC) * nblk + j
                        nc.sync.dma_start(
                            hT_blk[:, k, SC * j:SC * (j + 1)], ag1_3d[r, k, :, :]
                        )
                a_sb = pers.tile([128, ICH, TBLK], BF16, tag="a_sb")
                for mp in range(ICH):
                    gps = ps_f.tile([128, TBLK], F32, space="PSUM", tag="gps")
                    ups = ps_f.tile([128, TBLK], F32, space="PSUM", tag="ups")
                    for k in range(HC):
                        wg = wstream.tile([128, 128], BF16, tag="wg")
                        nc.sync.dma_start(
                            wg[:], w1T[128 * k:128 * (k + 1), 128 * mp:128 * (mp + 1)]
                        )
                        wu = wstream.tile([128, 128], BF16, tag="wu")
                        nc.sync.dma_start(
                            wu[:],
                            w1T[128 * k:128 * (k + 1),
                                I + 128 * mp:I + 128 * (mp + 1)],
                        )
                        nc.tensor.matmul(
                            out=gps[:], lhsT=wg[:], rhs=hT_blk[:, k, :],
                            start=(k == 0), stop=(k == HC - 1),
                        )
                        nc.tensor.matmul(
                            out=ups[:], lhsT=wu[:], rhs=hT_blk[:, k, :],
                            start=(k == 0), stop=(k == HC - 1),
                        )
                    sg = work.tile([128, TBLK], BF16, tag="sg")
                    nc.scalar.activation(
                        out=sg[:], in_=gps[:],
                        func=mybir.ActivationFunctionType.Silu,
                    )
                    nc.vector.tensor_mul(a_sb[:, mp, :], sg[:], ups[:])

                # down proj in two 512-wide output halves (PSUM budget)
                for halfo in range(2):
                    yps = []
                    for mt in range(TBLK // 128):
                        yp = ps_f.tile(
                            [128, 512], F32, space="PSUM",
                            tag=f"yps{mt}", name=f"yp{mt}",
                        )
                        yps.append(yp)
                    for kc in range(ICH):
                        w2t = wstream.tile([128, 512], BF16, tag="w2t")
                        nc.sync.dma_start(
                            w2t[:],
                            w2T[128 * kc:128 * (kc + 1),
                                512 * halfo:512 * (halfo + 1)],
                        )
                        for mt in range(TBLK // 128):
                            nc.tensor.matmul(
                                out=yps[mt][:],
                                lhsT=a_sb[:, kc, 128 * mt:128 * (mt + 1)],
                                rhs=w2t[:],
                                start=(kc == 0), stop=(kc == ICH - 1),
                            )
                    for mt in range(TBLK // 128):
                        yw = work.tile([128, 512], F32, tag="yw")
                        nc.vector.tensor_scalar_mul(
                            out=yw[:], in0=yps[mt][:],
                            scalar1=wcol[:, (TBLK // 128) * nblk + mt:
                                         (TBLK // 128) * nblk + mt + 1],
                        )
                        row0 = TBLK * nblk + 128 * mt
                        nc.sync.dma_start(
                            outAcc[row0:row0 + 128, 512 * halfo:512 * (halfo + 1)],
                            yw[:],
                        )
            ps_f_cm.__exit__(None, None, None)

            # ---------- combine across experts ----------
            nc.gpsimd.collective_compute(
                "ReduceScatter", mybir.AluOpType.add,
                replica_groups=[list(range(NCORES))],
                ins=[outAcc[:, :]], outs=[rs_out[:, :]],
            )
            ob = pers.tile([128, 2, H], F32, tag="ob")
            nc.sync.dma_start(
                ob[:], rs_out[:, :].rearrange("(c p) f -> p c f", p=128)
            )
            nc.sync.dma_start(
                moe_out[:, :].rearrange("(c p) f -> p c f", p=128), ob[:]
            )

    nc.compile()
    return nc


def _prep_inputs(hidden_states, positions, w_qkv, w_o, router_w, ws, w2s,
                 ln1_w, ln2_w):
    x = np.asarray(hidden_states, np.float32)[0]          # [S, H]
    pos = np.asarray(positions).astype(np.int64)
    w_qkv = np.asarray(w_qkv, np.float32)
    w_o = np.asarray(w_o, np.float32)
    router_w = np.asarray(router_w, np.float32)
    ws = np.asarray(ws, np.float32)
    w2s = np.asarray(w2s, np.float32)
    ln1 = np.asarray(ln1_w, np.float32)
    ln2 = np.asarray(ln2_w, np.float32)

    scale = HD ** -0.5
    wq = (w_qkv[: NH * HD] * ln1[None, :] * scale).T.copy()      # [H, 1024]
    wk = (w_qkv[NH * HD: NH * HD + NKV * HD] * ln1[None, :]).T.copy()
    wv = (w_qkv[NH * HD + NKV * HD:] * ln1[None, :]).T.copy()
    woT = np.ascontiguousarray(w_o.T)                            # [1024, H]
    routerT = (router_w * ln2[None, :]).T.copy()                 # [H, E]

    bf = ml_dtypes.bfloat16
    in_maps = []
    for c in range(NCORES):
        lo = SC * c - WIN
        xT_halo = np.zeros((H, HALO), np.float32)
        src_lo = max(lo, 0)
        xT_halo[:, src_lo - lo:] = x[src_lo: SC * c + SC].T
        # additive mask from the real positions input
        qpos = pos[SC * c: SC * c + SC]                          # [SC]
        kpos = lo + np.arange(HALO)                              # may be < 0
        ok = (kpos[:, None] <= qpos[None, :]) & \
             (qpos[None, :] - kpos[:, None] < WIN) & (kpos[:, None] >= 0)
        maskT = np.where(ok, 0.0, -1e9).astype(np.float32)
        in_maps.append({
            "xT": xT_halo,
            "maskT": maskT,
            "wqT": wq, "wkT": wk, "wvT": wv, "woT": woT,
            "routerT": routerT,
            "w1T": (ws[c] * ln2[None, :]).T.astype(bf).copy(),
            "w2T": np.ascontiguousarray(w2s[c].T).astype(bf),
            "cidx": np.full((128, 2), float(c), np.float32),
            "identf": np.eye(128, dtype=np.float32),
            "onesf": np.ones((128, 128), np.float32),
            "onesb": np.ones((128, 1), ml_dtypes.bfloat16),
            "epsi": np.full((1, 1), 1e-5, np.float32),
        })
    return in_maps


def _run(inputs, trace=False):
    if "nc" not in _CACHE:
        _CACHE["nc"] = _build_module()
    nc = _CACHE["nc"]
    in_maps = _prep_inputs(**inputs)
    res = run_bass_kernel_spmd(
        nc, in_maps, core_ids=list(range(NCORES)), trace=trace
    )
    outs = res.results
    out = np.concatenate([outs[c]["moe_out"] for c in range(NCORES)], 0)[None]
    res2 = np.concatenate(
        [outs[c]["res2T"].T for c in range(NCORES)], 0
    )[None]
    return (out.astype(np.float32), res2.astype(np.float32)), res


def kernel(**inputs):
    (out, res2), _ = _run(inputs, trace=False)
    return out, res2

